# revision 1
# baseline (speedup 1.0000x reference)
"""Trainium2 Bass kernel for nn_Attention_linearCombination.

out = sum_i softmax_i(tanh(x_i @ W_att_i + b_att_i) @ v) * (x_i @ W_tr_i + b_tr_i)

Sharding: data-parallel over the batch dim (16384 -> 8 cores x 2048 rows);
weights replicated. Per core, 16 tiles of 128 rows:
  - x tiles cast-loaded fp32->bf16 by SWDGE DMA, transposed by the xbar
    DMA-transpose into [128, 8, 128] (k-major) for the PE.
  - PE: per branch, 8 accumulating bf16 matmuls for x@W_att (N=256) and
    x@W_tr (N=512) with the transposed x chunk as the stationary operand;
    same-PSUM-bank matmul groups are kept contiguous (bank switches cost a
    pipeline re-setup). b_tr is added via a K=1 ones-row matmul; b_att is
    added on DVE (broadcast once at setup) to keep PE work minimal.
  - tanh via sigmoid identity (tanh(a) = 2*sigmoid(2a) - 1; Tanh ACT table
    crashes the device on this runtime), logits l_i = 2*(sigmoid(2a) @ v) - sum(v).
  - softmax via sigma-ratio (e^l = sig(l)/(1-sig(l))) to stay on the
    sigmoid ACT table set (exp lives in a different set -> reload thrash;
    tensor_tensor_reduce also crashes the device -> separate mul+reduce).
  - combine: unscaled PSUM evacuation on ACT, then per-partition-scaled
    copies and two DVE adds; loads prefetch 2 tiles ahead (SWDGE/Pool ring),
    transposes 1 tile ahead (SP HWDGE ring), stores on the Pool ring so no
    HWDGE wait ever blocks the transpose sequencer.
"""
import numpy as np

import concourse.bass as bass
import concourse.bacc as bacc
import concourse.mybir as mybir
import concourse.tile as tile
from concourse.bass_utils import run_bass_kernel_spmd

F32 = mybir.dt.float32
BF16 = mybir.dt.bfloat16
AF = mybir.ActivationFunctionType
OP = mybir.AluOpType

B = 16384
D = 1024
INT = 256
OUT = 512
NB = 3
NCORES = 8
B_LOC = B // NCORES
KC = D // 128
N_TILES = B_LOC // 128

_CACHE = {}


def _build_nc(repeat=1, loop_repeat=1):
    nc = bacc.Bacc(None, target_bir_lowering=False, num_swdge_queues=2)
    xs = [nc.dram_tensor(f"x{i+1}", [B_LOC, D], F32, kind="ExternalInput") for i in range(NB)]
    Was = [nc.dram_tensor(f"W_att{i+1}", [D, INT], F32, kind="ExternalInput") for i in range(NB)]
    bas = [nc.dram_tensor(f"b_att{i+1}", [1, INT], F32, kind="ExternalInput") for i in range(NB)]
    Wts = [nc.dram_tensor(f"W_tr{i+1}", [D, OUT], F32, kind="ExternalInput") for i in range(NB)]
    bts = [nc.dram_tensor(f"b_tr{i+1}", [1, OUT], F32, kind="ExternalInput") for i in range(NB)]
    v = nc.dram_tensor("v", [INT, 1], F32, kind="ExternalInput")
    out = nc.dram_tensor("out", [B_LOC, OUT], F32, kind="ExternalOutput")

    with tile.TileContext(nc) as tc:
        with (
            tc.tile_pool(name="wpool", bufs=1) as wpool,
            tc.tile_pool(name="work", bufs=4) as pool,
            tc.tile_pool(name="xpool", bufs=6) as xpool,
            tc.tile_pool(name="psum", bufs=4, space="PSUM") as psum,
            tc.tile_pool(name="ptr", bufs=4, space="PSUM") as ptrpool,
        ):
            # ---- one-time setup: weights to SBUF as bf16, k-chunked ----
            Wa_sb, Wt_sb, ba_sb, bt_sb = [], [], [], []
            for i in range(NB):
                wa = wpool.tile([128, KC, INT], BF16, tag=f"wa{i}")
                nc.gpsimd.dma_start(out=wa[:], in_=Was[i].rearrange("(c p) n -> p c n", p=128))
                Wa_sb.append(wa)
                wt = wpool.tile([128, KC, OUT], BF16, tag=f"wt{i}")
                nc.gpsimd.dma_start(out=wt[:], in_=Wts[i].rearrange("(c p) n -> p c n", p=128))
                Wt_sb.append(wt)
                bav = wpool.tile([1, INT], BF16, tag=f"ba{i}")
                nc.gpsimd.dma_start(out=bav[:], in_=bas[i][:])
                ba_sb.append(bav)
                btv = wpool.tile([1, OUT], BF16, tag=f"bt{i}")
                nc.gpsimd.dma_start(out=btv[:], in_=bts[i][:])
                bt_sb.append(btv)
            ones16 = wpool.tile([1, 128], BF16, tag="ones16")
            nc.vector.memset(ones16[:], 1.0)
            ones32 = wpool.tile([1, 128], F32, tag="ones32")
            nc.vector.memset(ones32[:], 1.0)

            # b_att broadcast to all partitions (K=1 matmul, one-time) so the
            # per-tile bias add runs on DVE instead of PE (PE is the bottleneck)
            ba_rep = []
            for i in range(NB):
                p_b = psum.tile([128, INT], F32, tag="att")
                nc.tensor.matmul(p_b[:], lhsT=ones16[:], rhs=ba_sb[i][:], start=True, stop=True)
                bar = wpool.tile([128, INT], F32, tag=f"barep{i}")
                nc.scalar.activation(bar[:], p_b[:], AF.Copy)
                ba_rep.append(bar)

            # v broadcast to all partitions via K=1 fp32 matmul
            v_row = wpool.tile([1, INT], F32, tag="vrow")
            nc.sync.dma_start(out=v_row[:], in_=v.rearrange("a b -> b a"))
            p_v = psum.tile([128, INT], F32, tag="att")
            nc.tensor.matmul(p_v[:], lhsT=ones32[:], rhs=v_row[:], start=True, stop=True)
            v2_rep = wpool.tile([128, INT], F32, tag="v2rep")
            nc.scalar.activation(v2_rep[:], p_v[:], AF.Copy, scale=2.0)
            vsum = wpool.tile([128, 1], F32, tag="vsum")
            nc.vector.reduce_sum(vsum[:], p_v[:], axis=mybir.AxisListType.X)

            # ---- main loop over 128-row tiles, software-pipelined ----
            # loads (SWDGE/Pool ring) run 2 tiles ahead, transposes (SP HWDGE
            # ring) 1 tile ahead, stores go on the Pool ring so they never
            # block the SP sequencer (HWDGE waits stall the issuing ring).
            import contextlib
            loop_cm = tc.For_i(0, loop_repeat, 1) if loop_repeat > 1 else contextlib.nullcontext()
            with loop_cm:
              xb_q, xT_q = {}, {}

              def issue_loads(t):
                  for i in range(NB):
                      xb = xpool.tile([128, D], BF16, tag=f"xb{i}")
                      nc.gpsimd.dma_start(out=xb[:], in_=xs[i][t * 128:(t + 1) * 128, :])
                      xb_q[(t, i)] = xb

              def issue_transposes(t):
                  for i in range(NB):
                      xT = xpool.tile([128, KC, 128], BF16, tag=f"xT{i}")
                      nc.sync.dma_start(out=xT[:], in_=xb_q.pop((t, i))[:], transpose=True)
                      xT_q[(t, i)] = xT

              tiles = [tt for _ in range(repeat) for tt in range(N_TILES)]
              issue_loads(tiles[0])
              if len(tiles) > 1:
                  issue_loads(tiles[1])
              issue_transposes(tiles[0])
              for ti, t in enumerate(tiles):
                if ti + 2 < len(tiles):
                    issue_loads(tiles[ti + 2])
                if ti + 1 < len(tiles):
                    issue_transposes(tiles[ti + 1])
                l3 = pool.tile([128, 4], F32, tag="l3")
                xTs = [xT_q.pop((t, i)) for i in range(NB)]
                # same-PSUM-bank matmuls stay contiguous: all att groups first,
                # then all tr groups (bank switches cost a pipeline re-setup)
                p_atts = []
                for i in range(NB):
                    p_att = psum.tile([128, INT], F32, tag="att")
                    for c in range(KC):
                        nc.tensor.matmul(p_att[:], lhsT=xTs[i][:, c, :], rhs=Wa_sb[i][:, c, :],
                                         start=(c == 0), stop=(c == KC - 1))
                    p_atts.append(p_att)
                p_trs = []
                for i in range(NB):
                    p_tr = ptrpool.tile([128, OUT], F32, tag="tr")
                    for c in range(KC):
                        nc.tensor.matmul(p_tr[:], lhsT=xTs[i][:, c, :], rhs=Wt_sb[i][:, c, :],
                                         start=(c == 0), stop=False)
                    nc.tensor.matmul(p_tr[:], lhsT=ones16[:], rhs=bt_sb[i][:], start=False, stop=True)
                    p_trs.append(p_tr)
                ptrs = []
                for i in range(NB):
                    # evacuate PSUM immediately (unscaled) so the bank frees
                    # without waiting for the softmax chain
                    traw = pool.tile([128, OUT], F32, tag=f"traw{i}")
                    nc.scalar.activation(traw[:], p_trs[i][:], AF.Copy)
                    ptrs.append(traw)

                    ab = pool.tile([128, INT], F32, tag="ab")
                    nc.vector.tensor_add(ab[:], p_atts[i][:], ba_rep[i][:])
                    sgh = pool.tile([128, INT], F32, tag="sgh")
                    nc.scalar.activation(sgh[:], ab[:], AF.Sigmoid, scale=2.0)
                    prod = pool.tile([128, INT], F32, tag="prod")
                    nc.vector.tensor_mul(prod[:], sgh[:], v2_rep[:])
                    raw = pool.tile([128, 1], F32, tag="raw")
                    nc.vector.reduce_sum(raw[:], prod[:], axis=mybir.AxisListType.X)
                    nc.vector.tensor_scalar(l3[:, i:i + 1], raw[:], vsum[:], None, OP.subtract)

                sg3 = pool.tile([128, 4], F32, tag="sg3")
                nc.scalar.activation(sg3[:, 0:NB], l3[:, 0:NB], AF.Sigmoid)
                u3 = pool.tile([128, 4], F32, tag="u3")
                nc.vector.tensor_scalar(u3[:, 0:NB], sg3[:, 0:NB], -1.0, 1.0, OP.mult, OP.add)
                w3 = pool.tile([128, 4], F32, tag="w3")
                nc.vector.reciprocal(w3[:, 0:NB], u3[:, 0:NB])
                r3 = pool.tile([128, 4], F32, tag="r3")
                nc.vector.tensor_mul(r3[:, 0:NB], sg3[:, 0:NB], w3[:, 0:NB])
                ssum = pool.tile([128, 1], F32, tag="ssum")
                nc.vector.reduce_sum(ssum[:], r3[:, 0:NB], axis=mybir.AxisListType.X)
                rs = pool.tile([128, 1], F32, tag="rs")
                nc.vector.reciprocal(rs[:], ssum[:])
                s3 = pool.tile([128, 4], F32, tag="s3")
                nc.vector.tensor_scalar_mul(s3[:, 0:NB], r3[:, 0:NB], rs[:])

                t0 = pool.tile([128, OUT], F32, tag="t0")
                t1 = pool.tile([128, OUT], F32, tag="t1")
                t2 = pool.tile([128, OUT], F32, tag="t2")
                for i, tt in enumerate([t0, t1, t2]):
                    nc.scalar.activation(tt[:], ptrs[i][:], AF.Copy, scale=s3[:, i:i + 1])
                a01 = pool.tile([128, OUT], F32, tag="a01")
                nc.vector.tensor_add(a01[:], t0[:], t1[:])
                acc = pool.tile([128, OUT], F32, tag="acc")
                nc.vector.tensor_add(acc[:], a01[:], t2[:])
                nc.gpsimd.dma_start(out=out[t * 128:(t + 1) * 128, :], in_=acc[:])
    nc.compile()
    return nc


LAST_RESULTS = None


def kernel(**inputs) -> np.ndarray:
    if "nc" not in _CACHE:
        _CACHE["nc"] = _build_nc()
    nc = _CACHE["nc"]

    shared = {}
    for i in range(NB):
        for k in (f"W_att{i+1}", f"b_att{i+1}", f"W_tr{i+1}", f"b_tr{i+1}"):
            shared[k] = np.ascontiguousarray(np.asarray(inputs[k], dtype=np.float32))
    shared["v"] = np.ascontiguousarray(np.asarray(inputs["v"], dtype=np.float32))

    in_maps = []
    for c in range(NCORES):
        m = dict(shared)
        for i in range(NB):
            m[f"x{i+1}"] = np.ascontiguousarray(
                np.asarray(inputs[f"x{i+1}"], dtype=np.float32)[c * B_LOC:(c + 1) * B_LOC]
            )
        in_maps.append(m)

    res = run_bass_kernel_spmd(nc, in_maps, core_ids=list(range(NCORES)))
    global LAST_RESULTS
    LAST_RESULTS = res
    return np.concatenate([r["out"] for r in res.results], axis=0)

